# revision 6
# baseline (speedup 1.0000x reference)
"""Trainium2 Bass kernel for nn_MixquantLinear: O = ((dequant4(V) * S) @ dequant4(U)).T.

Output O is [4096, 4096] fp32, built purely from the GPTQ-quantized weights
(the activation input `x` is dead code in the reference). Sharding: 4 slices
over output rows (o) x 2 over output cols (i) -> 8 cores, no collectives;
host concatenates the blocks.

Key idea: the contraction index r is relabeled r' = j*128 + p with r = 8p + j
(word p, nibble j). Under this permutation a single full-width shift+mask of
the packed U words [128 words, O_SL] directly yields k-tile j of the lhsT
operand in [r, o] layout -- no PE transposes, no PSUM round trips. The V side
and all scale/zero tables are permuted to match (host does layout-only
slicing/transpose/gather of words and tables; all dequant arithmetic is
on-device).

Per core:
  - U: 8 wide shifts unpack k-tiles; dequant via two wide tensor_tensor ops
    against [128, O_SL] broadcast tables (zeros+1, scales) built with one
    K=128 indicator matmul (partition broadcast on the PE).
  - V: 8 wide shifts per strip unpack all k-tiles at once; per-group fused
    affine (q*a + b) with a = scale*S, b = -(z+1)*scale*S, split across
    DVE/ACT/GPSIMD engines.
  - fp16 matmuls (k-tiles of 128, N=512) accumulating fp32 in PSUM; first
    wave is k-layered to chase strip-0 dequant. Output flushed as fp16
    (rounding ~2^-11, far under tolerance), halving flush + DMA cost.
Host-side work is layout-only (slicing/permuting packed int32 words and fp32
tables, concatenating outputs; final fp16->fp32 cast).
"""

import numpy as np

import concourse.bass as bass
import concourse.mybir as mybir
import concourse.tile as tile
from concourse import bacc
from concourse.bass_utils import run_bass_kernel_spmd

IN_SIZE = 4096
OUT_SIZE = 4096
RANK = 1024
PACK = 8
P_O = 4
P_I = 2
O_SL = OUT_SIZE // P_O    # 1024
I_SL = IN_SIZE // P_I     # 2048
N_CORES = P_O * P_I
J = RANK // 128           # 8 k-tiles
N_STRIPS = 2
STRIP = I_SL // N_STRIPS  # 1024
GPS = 16                  # V groups per core slice (I_SL / 128)

F16 = mybir.dt.float16
F32 = mybir.dt.float32
I32 = mybir.dt.int32
Alu = mybir.AluOpType
Act = mybir.ActivationFunctionType

_NC_CACHE = None
TRACE = False
LAST_RESULTS = None


def _build_nc():
    nc = bacc.Bacc("TRN2", target_bir_lowering=False)

    qut = nc.dram_tensor("qut", [128, O_SL], I32, kind="ExternalInput")
    qvt = nc.dram_tensor("qvt", [128, N_STRIPS * J * 128], I32, kind="ExternalInput")
    dma_u = nc.dram_tensor("dma_u", [8, 1152], I32, kind="ExternalInput")
    dma_v = nc.dram_tensor("dma_v", [128, 216], I32, kind="ExternalInput")
    out = nc.dram_tensor("out", [O_SL, I_SL], F16, kind="ExternalOutput")

    cnt = {"vup": 0, "vaff": 0, "uup": 0, "utt": 0}

    with tile.TileContext(nc) as tc:
        with (
            tc.tile_pool(name="const", bufs=1) as cp,
            tc.tile_pool(name="nib", bufs=3) as nibp,
            tc.tile_pool(name="outsb", bufs=4) as outp,
        ):
            qut_sb = cp.tile([128, O_SL], I32, tag="qut")
            qvt_sb = cp.tile([128, N_STRIPS * J * 128], I32, tag="qvt")
            dmu_sb = cp.tile([8, 1152], I32, tag="dmu")
            dmv_sb = cp.tile([128, 216], I32, tag="dmv")
            rhs = [cp.tile([128, J * STRIP], F16, tag=f"rhs{s}", name=f"rhs{s}")
                   for s in range(N_STRIPS)]
            nibv = [cp.tile([128, J * STRIP], I32, tag=f"nv{s}", name=f"nv{s}")
                    for s in range(N_STRIPS)]
            lhsT = cp.tile([128, J * O_SL], F16, tag="lhsT")
            zu8 = cp.tile([8, O_SL], I32, tag="zu8")
            rhs_bc = cp.tile([128, 2 * O_SL], F16, tag="rhsbc")
            zub_b = cp.tile([128, O_SL], F16, tag="zubb")
            su_b = cp.tile([128, O_SL], F16, tag="sub")
            zv_u = cp.tile([128, J * GPS], I32, tag="zvu")
            av = cp.tile([128, J * GPS], F32, tag="av")
            bvn = cp.tile([128, J * GPS], F32, tag="bvn")

            # views into the consolidated DMA payloads
            e_sb = dmu_or = None
            qzu_sb = dmu_sb[:, 0:128]
            su8 = dmu_sb[:, 128:1152].bitcast(F32)        # [8, 1024]
            e_sb = dmv_sb[:, 0:64].bitcast(F16)           # [128, 128]
            qzv_t = dmv_sb[:, 64:80]                      # [128, 16]
            svt_p = dmv_sb[:, 80:208].bitcast(F32)        # [128, 128]
            s_p = dmv_sb[:, 208:216].bitcast(F32)         # [128, 8]

            # ---- input DMAs, priority order ----
            nc.sync.dma_start(out=dmu_sb[:], in_=dma_u[:])
            nc.sync.dma_start(out=dmv_sb[:], in_=dma_v[:])
            nc.sync.dma_start(out=qut_sb[:], in_=qut[:])
            for s in range(N_STRIPS):
                nc.sync.dma_start(out=qvt_sb[:, s * 1024:(s + 1) * 1024],
                                  in_=qvt[:, s * 1024:(s + 1) * 1024])

            # ---- U tables: unpack zeros, broadcast (zu+1)|su over partitions ----
            nc.vector.memset(rhs_bc[:], 0.0)
            zu8_r = zu8[:].rearrange("p (w q) -> p w q", q=PACK)
            for jo in range(PACK):
                nc.vector.tensor_scalar(
                    out=zu8_r[:, :, jo], in0=qzu_sb, scalar1=4 * jo, scalar2=15,
                    op0=Alu.logical_shift_right, op1=Alu.bitwise_and)
            nc.vector.tensor_scalar(
                out=rhs_bc[0:8, 0:O_SL], in0=zu8[:], scalar1=1.0, scalar2=1.0,
                op0=Alu.mult, op1=Alu.add)
            nc.gpsimd.tensor_copy(rhs_bc[0:8, O_SL:2 * O_SL], su8)

            with tc.tile_pool(name="bc", bufs=4, space="PSUM") as bps:
                for q in range(4):
                    pt = bps.tile([128, 512], F32, tag="bc", name="bc")
                    nc.tensor.matmul(pt[:], e_sb, rhs_bc[:, q * 512:(q + 1) * 512],
                                     start=True, stop=True)
                    dst = (zub_b[:, q * 512:(q + 1) * 512] if q < 2
                           else su_b[:, (q - 2) * 512:(q - 1) * 512])
                    nc.scalar.copy(dst, pt[:])

            # ---- V tables: unpack zeros, a = sv*S, b = -(zv+1)*a ----
            for j in range(J):
                nc.vector.tensor_scalar(
                    out=zv_u[:, j * GPS:(j + 1) * GPS], in0=qzv_t, scalar1=4 * j,
                    scalar2=15, op0=Alu.logical_shift_right, op1=Alu.bitwise_and)
            for j in range(J):
                nc.vector.tensor_scalar(
                    out=av[:, j * GPS:(j + 1) * GPS], in0=svt_p[:, j * GPS:(j + 1) * GPS],
                    scalar1=s_p[:, j:j + 1], scalar2=None, op0=Alu.mult)
            nc.vector.tensor_scalar(
                out=bvn[:], in0=zv_u[:], scalar1=1.0, scalar2=-1.0,
                op0=Alu.add, op1=Alu.mult)
            nc.vector.tensor_tensor(bvn[:], bvn[:], av[:], Alu.mult)

            # ---- U dequant: per k-tile j, one wide shift + two wide tts ----
            for j in range(J):
                lj = lhsT[:, j * O_SL:(j + 1) * O_SL]
                nibu = nibp.tile([128, O_SL], I32, tag="nibu", name="nibu")
                cnt["uup"] += 1
                nc.vector.tensor_scalar(
                    out=nibu[:], in0=qut_sb[:], scalar1=4 * j, scalar2=15,
                    op0=Alu.logical_shift_right, op1=Alu.bitwise_and)
                # int32 - fp16 mixed tt only works on DVE; fp16 mult can go GPS
                nc.vector.tensor_tensor(lj, nibu[:], zub_b[:], Alu.subtract)
                cnt["utt"] += 1
                nc.gpsimd.tensor_tensor(lj, lj, su_b[:], Alu.mult)

            # ---- V dequant per strip ----
            def v_unpack(s, jhalf):
                j0 = jhalf * (J // 2)
                src = qvt_sb[:, s * 1024 + j0 * 128:
                             s * 1024 + (j0 + J // 2) * 128].rearrange(
                    "p (j w) -> p j w", j=J // 2)
                dst = nibv[s][:, j0 * STRIP:(j0 + J // 2) * STRIP].rearrange(
                    "p (j w q) -> p j w q", j=J // 2, q=PACK)
                for jj in range(PACK):
                    cnt["vup"] += 1
                    nc.vector.tensor_scalar(
                        out=dst[:, :, :, jj], in0=src, scalar1=4 * jj, scalar2=15,
                        op0=Alu.logical_shift_right, op1=Alu.bitwise_and)

            def v_affine(s, jlo=0, jhi=J):
                for j in range(jlo, jhi):
                    for g in range(STRIP // 128):
                        col = j * GPS + s * (STRIP // 128) + g
                        o_ap = rhs[s][:, j * STRIP + g * 128:j * STRIP + (g + 1) * 128]
                        i_ap = nibv[s][:, j * STRIP + g * 128:j * STRIP + (g + 1) * 128]
                        k = cnt["vaff"]
                        cnt["vaff"] += 1
                        if k % 8 < 5:
                            nc.scalar.activation(
                                o_ap, i_ap, Act.Identity,
                                bias=bvn[:, col:col + 1], scale=av[:, col:col + 1])
                        else:
                            nc.vector.tensor_scalar(
                                out=o_ap, in0=i_ap, scalar1=av[:, col:col + 1],
                                scalar2=bvn[:, col:col + 1], op0=Alu.mult, op1=Alu.add)

            v_unpack(0, 0)
            v_affine(0, 0, J // 2)
            v_unpack(0, 1)
            v_affine(0, J // 2, J)

            # ---- matmul waves ----
            def mm(pt, j, m, s, h, start, stop):
                nc.tensor.matmul(
                    pt[:],
                    lhsT[:, j * O_SL + m * 128:j * O_SL + (m + 1) * 128],
                    rhs[s][:, j * STRIP + h * 512:j * STRIP + (h + 1) * 512],
                    start=start, stop=stop)

            with tc.tile_pool(name="mps", bufs=8, space="PSUM") as mps:
                # wave A: strip 0, h 0 -- k-layered to chase dequant
                wa = [mps.tile([128, 512], F32, tag="mm", name="mmps")
                      for _ in range(8)]
                for j in range(J):
                    for m in range(8):
                        mm(wa[m], j, m, 0, 0, j == 0, j == J - 1)

                # strip-1 dequant (overlaps wave A on non-PE engines)
                v_unpack(1, 0)
                v_unpack(1, 1)

                # flush wave A (ACT), dma out
                for m in range(8):
                    ot = outp.tile([128, 512], F16, tag="ot", name="ot")
                    (nc.scalar.copy if m % 2 == 0 else nc.vector.tensor_copy)(
                        ot[:], wa[m][:])
                    nc.sync.dma_start(
                        out=out[m * 128:(m + 1) * 128, 0:512], in_=ot[:])

                v_affine(1)

                # wave B: strip 0, h 1 -- m-grouped, inline flush
                for m in range(8):
                    tb = mps.tile([128, 512], F32, tag="mm", name="mmps")
                    for j in range(J):
                        mm(tb, j, m, 0, 1, j == 0, j == J - 1)
                    ot = outp.tile([128, 512], F16, tag="ot", name="ot")
                    (nc.vector.tensor_copy if m % 2 == 0 else nc.scalar.copy)(
                        ot[:], tb[:])
                    nc.sync.dma_start(
                        out=out[m * 128:(m + 1) * 128, 512:1024], in_=ot[:])

                # waves C+D: strip 1, h 0/1 paired per m
                for m in range(8):
                    tcx = mps.tile([128, 512], F32, tag="mm", name="mmps")
                    tdx = mps.tile([128, 512], F32, tag="mm", name="mmps")
                    for j in range(J):
                        mm(tcx, j, m, 1, 0, j == 0, j == J - 1)
                        mm(tdx, j, m, 1, 1, j == 0, j == J - 1)
                    ot = outp.tile([128, 1024], F16, tag="ot2", name="ot2")
                    nc.scalar.copy(ot[:, 0:512], tcx[:])
                    nc.vector.tensor_copy(ot[:, 512:1024], tdx[:])
                    nc.sync.dma_start(
                        out=out[m * 128:(m + 1) * 128, 1024:2048], in_=ot[:])

    nc.compile()
    return nc


def _host_prep(qweight_V, qzeros_V, scales_V, qweight_U, qzeros_U, scales_U, S):
    """Layout-only host prep: slice/permute packed int32 words + fp32 tables."""
    p = np.arange(128)
    rperm = (8 * p[:, None] + np.arange(8)[None, :]).reshape(-1)  # [p*8+j] -> r
    E128 = np.zeros((128, 128), dtype=np.float16)
    E128[np.arange(128) // 16, np.arange(128)] = 1.0
    e_i32 = np.ascontiguousarray(E128).view(np.int32)             # [128, 64]
    s_p = np.ascontiguousarray(S.reshape(128, 8))                 # S[8p+j]

    in_maps = []
    for c in range(N_CORES):
        a, b = divmod(c, P_I)
        qut_h = np.ascontiguousarray(qweight_U[:, a * O_SL:(a + 1) * O_SL])
        qzu = qzeros_U[:, a * (O_SL // 8):(a + 1) * (O_SL // 8)]   # [8, 128]
        su8 = scales_U[:, a * O_SL:(a + 1) * O_SL]                 # [8, 1024]
        dma_u = np.ascontiguousarray(
            np.concatenate([qzu, su8.view(np.int32)], axis=1))

        qvT = qweight_V[b * 256:(b + 1) * 256, :].T                # [1024, 256]
        Aq = qvT[rperm].reshape(128, 8, 2, 128)                    # [p, j, s, iw]
        qvt_h = np.ascontiguousarray(
            Aq.transpose(0, 2, 1, 3).reshape(128, 2048))

        svs = scales_V[b * 16:(b + 1) * 16, :]                     # [16, 1024]
        svt_p = np.ascontiguousarray(svs.T[rperm].reshape(128, 128))
        qzv_t = np.ascontiguousarray(qzeros_V[b * 16:(b + 1) * 16, :].T)
        dma_v = np.ascontiguousarray(np.concatenate(
            [e_i32, qzv_t, svt_p.view(np.int32), s_p.view(np.int32)], axis=1))

        in_maps.append({
            "qut": qut_h, "qvt": qvt_h, "dma_u": dma_u, "dma_v": dma_v,
        })
    return in_maps


def kernel(x, qweight_V, qzeros_V, scales_V, g_idx_V,
           qweight_U, qzeros_U, scales_U, g_idx_U, S, **_unused):
    global _NC_CACHE, LAST_RESULTS
    qweight_V = np.asarray(qweight_V, dtype=np.int32)
    qzeros_V = np.asarray(qzeros_V, dtype=np.int32)
    scales_V = np.asarray(scales_V, dtype=np.float32)
    qweight_U = np.asarray(qweight_U, dtype=np.int32)
    qzeros_U = np.asarray(qzeros_U, dtype=np.int32)
    scales_U = np.asarray(scales_U, dtype=np.float32)
    S = np.asarray(S, dtype=np.float32)

    if _NC_CACHE is None:
        _NC_CACHE = _build_nc()
    nc = _NC_CACHE

    in_maps = _host_prep(qweight_V, qzeros_V, scales_V,
                         qweight_U, qzeros_U, scales_U, S)
    res = run_bass_kernel_spmd(nc, in_maps, core_ids=list(range(N_CORES)), trace=TRACE)
    LAST_RESULTS = res

    O = np.empty((OUT_SIZE, IN_SIZE), dtype=np.float32)
    for c in range(N_CORES):
        a, b = divmod(c, P_I)
        O[a * O_SL:(a + 1) * O_SL, b * I_SL:(b + 1) * I_SL] = \
            res.results[c]["out"].astype(np.float32)
    return O
